# revision 3
# baseline (speedup 1.0000x reference)
"""Trainium2 Bass kernel for the vq_codebook problem.

reference math:
    xf = x.reshape(B, I); xf = xf / sum(xf, -1, keepdims=True)
    scores = einsum('bi,cin->bcn', xf, W)      # [B, C, N]
    out = one_hot(argmax(scores, -1), N)       # [B, C, N] float32

Design (v3 — bf16-hi/lo x, packed fp16-hi/lo w):
  * argmax over n is invariant to the positive per-row normalization and
    to any per-(b,c) constant, so the row-normalize step is skipped and
    x/w are centered by -0.5 on the host (halves rounding error).
  * The C=32 codebooks are independent -> shard C across the 8 cores
    (4 CMs per core).
  * Precision: x-0.5 = xh(bf16) + xl(bf16)  (~17-bit capture);
    w-0.5 = wh(fp16) + 2^-10 * wl(fp16)     (~26-bit capture, wl
    pre-scaled by 2^10 so its values stay in the fp16 normal range).
    Score = xh*wh + 2^-10*xh*wl + xl*wh  (+ dropped xl*wl ~ 2^-22).
    wh/wl are packed column-wise into one [I, 512] tensor so the two
    xh passes become a single matmul streaming 512 moving columns.
    The column constant 0.5*sum_i(w-0.5) (from undoing the centering;
    only its n-dependence matters for argmax) is precomputed exactly
    on the host and added on DVE.  Total score noise ~1.5e-4, ~40x
    below the smallest top-2 gap; fp32 PSUM accumulation with a 2-way
    k-split; all products of the 16-bit inputs are exact in fp32.
  * Argmax on DVE: segment reduce_max, then (score==max)*(64-n) ->
    reduce_max recovers the FIRST argmax index (ties break low like
    jnp.argmax), one-hot via is_equal against (64-n).

Per-core layout: xh/xl [I=16384, B=256] bf16, wq [I, 512] fp16
(cols 0:256 = wh for 4 CMs i-major, 256:512 = wl*2^10), out oh
[256, 256] fp32.  PE per k-chunk per b-tile: xh-chunk [128, 128b] x
wq-chunk [128, 512] plus xl-chunk [128, 128b] x wh-chunk [128, 256].
"""

from contextlib import ExitStack

import numpy as np
import ml_dtypes

import concourse.bacc as bacc
import concourse.bass as bass
import concourse.mybir as mybir
import concourse.tile as tile
from concourse import bass_utils

B = 256
I = 16384
C = 32
N = 64
N_CORES = 8
CPC = C // N_CORES          # CMs per core = 4
CN = CPC * N                # per-core score columns = 256
KC = 128                    # contraction chunk (partition dim)
NKC = I // KC               # 128 k-chunks
G = 8                       # k-chunks per DMA
P = 128
WLS = 1024.0                # wl pre-scale (2^10)

_compiled = None
LAST_RESULTS = None


def _build():
    nc = bacc.Bacc("TRN2", target_bir_lowering=False, debug=False,
                   num_devices=N_CORES)

    f32 = mybir.dt.float32
    f16 = mybir.dt.float16
    bf16 = mybir.dt.bfloat16

    xh_d = nc.dram_tensor("xh", [I, B], bf16, kind="ExternalInput").ap()
    xl_d = nc.dram_tensor("xl", [I, B], bf16, kind="ExternalInput").ap()
    wq_d = nc.dram_tensor("wq", [I, 2 * CN], f16, kind="ExternalInput").ap()
    corr_d = nc.dram_tensor("corr", [P, CN], f32, kind="ExternalInput").ap()
    rev_d = nc.dram_tensor("revio", [P, CN], f32, kind="ExternalInput").ap()
    oh_d = nc.dram_tensor("oh", [B, CN], f32, kind="ExternalOutput").ap()

    with tile.TileContext(nc) as tc:
        with ExitStack() as ctx:
            cpool = ctx.enter_context(tc.tile_pool(name="const", bufs=1))
            xhp = ctx.enter_context(tc.tile_pool(name="xhp", bufs=16))
            xlp = ctx.enter_context(tc.tile_pool(name="xlp", bufs=16))
            wp = ctx.enter_context(tc.tile_pool(name="wp", bufs=7))
            ppool = ctx.enter_context(tc.tile_pool(name="ps", bufs=1, space="PSUM"))
            dpool = ctx.enter_context(tc.tile_pool(name="dv", bufs=2))
            opool = ctx.enter_context(tc.tile_pool(name="ohp", bufs=2))

            rev_t = cpool.tile([P, CN], f32)
            nc.sync.dma_start(rev_t[:], rev_d[:])
            corr_t = cpool.tile([P, CN], f32)
            nc.sync.dma_start(corr_t[:], corr_d[:])

            # Per b-tile: 2 k-split accumulators [128, 512] for the xh
            # passes (cols 0:256 = xh*wh, 256:512 = xh*wl), and one
            # [128, 256] accumulator for the xl*wh pass.
            am = [[ppool.tile([P, 2 * CN], f32, tag=f"am{bt}{q}",
                              name=f"am{bt}{q}") for q in range(2)]
                  for bt in range(2)]
            al = [ppool.tile([P, CN], f32, tag=f"al{bt}", name=f"al{bt}")
                  for bt in range(2)]

            for it in range(NKC // G):
                xhg = xhp.tile([P, G, B], bf16)
                nc.gpsimd.dma_start(
                    xhg[:],
                    xh_d[it * G * KC:(it + 1) * G * KC, :]
                    .rearrange("(p g) j -> p g j", g=G))
                xlg = xlp.tile([P, G, B], bf16)
                nc.gpsimd.dma_start(
                    xlg[:],
                    xl_d[it * G * KC:(it + 1) * G * KC, :]
                    .rearrange("(p g) j -> p g j", g=G))
                wg = wp.tile([P, G, 2 * CN], f16)
                nc.sync.dma_start(
                    wg[:],
                    wq_d[it * G * KC:(it + 1) * G * KC, :]
                    .rearrange("(p g) j -> p g j", g=G))
                for g in range(G):
                    kc = it * G + g
                    q, pos = divmod(kc, NKC // 2)
                    for bt in range(2):
                        bs = slice(bt * P, (bt + 1) * P)
                        nc.tensor.matmul(
                            am[bt][q][:],
                            lhsT=xhg[:, g, bs], rhs=wg[:, g, :],
                            start=(pos == 0), stop=(pos == NKC // 2 - 1))
                        nc.tensor.matmul(
                            al[bt][:],
                            lhsT=xlg[:, g, bs], rhs=wg[:, g, 0:CN],
                            start=(kc == 0), stop=(kc == NKC - 1))

            for bt in range(2):
                # Combine; never two PSUM operands in one op.
                c0 = dpool.tile([P, 2 * CN], f32, tag="c0")
                nc.vector.tensor_copy(c0[:], am[bt][0][:])
                a1 = dpool.tile([P, 2 * CN], f32, tag="a1")
                nc.vector.tensor_add(a1[:], c0[:], am[bt][1][:])
                # sw = xh*wh + 2^-10 * xh*wl
                sw = dpool.tile([P, CN], f32, tag="sw")
                nc.vector.scalar_tensor_tensor(
                    sw[:], a1[:, CN:2 * CN], 1.0 / WLS, a1[:, 0:CN],
                    op0=mybir.AluOpType.mult,
                    op1=mybir.AluOpType.add)
                sx = dpool.tile([P, CN], f32, tag="sx")
                nc.vector.tensor_add(sx[:], sw[:], al[bt][:])
                s_t = dpool.tile([P, CN], f32, tag="s")
                nc.vector.tensor_add(s_t[:], sx[:], corr_t[:])

                s3 = s_t[:].rearrange("p (s j) -> p s j", s=CPC)
                maxs = dpool.tile([P, CPC], f32, tag="maxs")
                nc.vector.tensor_reduce(maxs[:], s3, mybir.AxisListType.X,
                                        mybir.AluOpType.max)
                t_t = dpool.tile([P, CN], f32, tag="tt")
                for s in range(CPC):
                    seg = slice(s * N, (s + 1) * N)
                    nc.vector.scalar_tensor_tensor(
                        t_t[:, seg], s_t[:, seg], maxs[:, s:s + 1],
                        rev_t[:, seg],
                        op0=mybir.AluOpType.is_equal,
                        op1=mybir.AluOpType.mult)
                m2 = dpool.tile([P, CPC], f32, tag="m2")
                nc.vector.tensor_reduce(
                    m2[:], t_t[:].rearrange("p (s j) -> p s j", s=CPC),
                    mybir.AxisListType.X, mybir.AluOpType.max)
                oh_t = opool.tile([P, CN], f32)
                for s in range(CPC):
                    seg = slice(s * N, (s + 1) * N)
                    nc.vector.tensor_scalar(
                        oh_t[:, seg], rev_t[:, seg], m2[:, s:s + 1], None,
                        op0=mybir.AluOpType.is_equal)
                nc.sync.dma_start(oh_d[bt * P:(bt + 1) * P, :], oh_t[:])

    nc.compile()
    return nc


def kernel(x, weights):
    global _compiled, LAST_RESULTS
    x = np.asarray(x, dtype=np.float32)
    w = np.asarray(weights, dtype=np.float32)

    xt = np.ascontiguousarray(x.reshape(B, I).T).astype(np.float64) - 0.5
    xh = xt.astype(ml_dtypes.bfloat16)
    xl = (xt - xh.astype(np.float64)).astype(ml_dtypes.bfloat16)
    xh = np.ascontiguousarray(xh)
    xl = np.ascontiguousarray(xl)
    j = np.arange(N, dtype=np.float32)
    revio = np.ascontiguousarray(
        np.tile(N - j, (P, CPC)).astype(np.float32))        # [128, 256]

    in_maps = []
    for c in range(N_CORES):
        wt = np.ascontiguousarray(
            w[c * CPC:(c + 1) * CPC].transpose(1, 0, 2).reshape(I, CN))
        wc = wt.astype(np.float64) - 0.5
        wh = wc.astype(np.float16)
        wl = ((wc - wh.astype(np.float64)) * WLS).astype(np.float16)
        wq = np.ascontiguousarray(
            np.concatenate([wh, wl], axis=1))               # [I, 512] fp16
        csum = 0.5 * wc.sum(axis=0)                         # [256] exact
        corr = np.ascontiguousarray(
            np.tile(csum.astype(np.float32), (P, 1)))       # [128, 256]
        in_maps.append({"xh": xh, "xl": xl, "wq": wq, "corr": corr,
                        "revio": revio})

    if _compiled is None:
        _compiled = _build()

    import os
    kwargs = {}
    if os.environ.get("KERNEL_TRACE"):
        kwargs = {"trace": True,
                  "tmpdir": os.environ.get("KERNEL_TRACE_DIR") or None}
    res = bass_utils.run_bass_kernel_spmd(
        _compiled, in_maps, core_ids=list(range(N_CORES)), **kwargs)
    LAST_RESULTS = res

    out = np.concatenate(
        [res.results[c]["oh"].reshape(B, CPC, N) for c in range(N_CORES)],
        axis=1)
    return np.ascontiguousarray(out.astype(np.float32))


# revision 4
# speedup vs baseline: 1.0197x; 1.0197x over previous
"""Trainium2 Bass kernel for the vq_codebook problem.

reference math:
    xf = x.reshape(B, I); xf = xf / sum(xf, -1, keepdims=True)
    scores = einsum('bi,cin->bcn', xf, W)      # [B, C, N]
    out = one_hot(argmax(scores, -1), N)       # [B, C, N] float32

Design (v3 — bf16-hi/lo x, packed fp16-hi/lo w):
  * argmax over n is invariant to the positive per-row normalization and
    to any per-(b,c) constant, so the row-normalize step is skipped and
    x/w are centered by -0.5 on the host (halves rounding error).
  * The C=32 codebooks are independent -> shard C across the 8 cores
    (4 CMs per core).
  * Precision: x-0.5 = xh(bf16) + xl(bf16)  (~17-bit capture);
    w-0.5 = wh(fp16) + 2^-10 * wl(fp16)     (~26-bit capture, wl
    pre-scaled by 2^10 so its values stay in the fp16 normal range).
    Score = xh*wh + 2^-10*xh*wl + xl*wh  (+ dropped xl*wl ~ 2^-22).
    wh/wl are packed column-wise into one [I, 512] tensor so the two
    xh passes become a single matmul streaming 512 moving columns.
    The column constant 0.5*sum_i(w-0.5) (from undoing the centering;
    only its n-dependence matters for argmax) is precomputed exactly
    on the host and added on DVE.  Total score noise ~1.5e-4, ~40x
    below the smallest top-2 gap; fp32 PSUM accumulation with a 2-way
    k-split; all products of the 16-bit inputs are exact in fp32.
  * Argmax on DVE: segment reduce_max, then (score==max)*(64-n) ->
    reduce_max recovers the FIRST argmax index (ties break low like
    jnp.argmax), one-hot via is_equal against (64-n).

Per-core layout: xh/xl [I=16384, B=256] bf16, wq [I, 512] fp16
(cols 0:256 = wh for 4 CMs i-major, 256:512 = wl*2^10), out oh
[256, 256] fp32.  PE per k-chunk per b-tile: xh-chunk [128, 128b] x
wq-chunk [128, 512] plus xl-chunk [128, 128b] x wh-chunk [128, 256].
"""

from contextlib import ExitStack

import numpy as np
import ml_dtypes

import concourse.bacc as bacc
import concourse.bass as bass
import concourse.mybir as mybir
import concourse.tile as tile
from concourse import bass_utils

B = 256
I = 16384
C = 32
N = 64
N_CORES = 8
CPC = C // N_CORES          # CMs per core = 4
CN = CPC * N                # per-core score columns = 256
KC = 128                    # contraction chunk (partition dim)
NKC = I // KC               # 128 k-chunks
G = 8                       # k-chunks per DMA
P = 128
WLS = 1024.0                # wl pre-scale (2^10)
XLS = 512.0                 # xl pre-scale (2^9)

_compiled = None
LAST_RESULTS = None


def _build():
    nc = bacc.Bacc("TRN2", target_bir_lowering=False, debug=False,
                   num_devices=N_CORES)

    f32 = mybir.dt.float32
    f16 = mybir.dt.float16
    bf16 = mybir.dt.bfloat16

    xh_d = nc.dram_tensor("xh", [I, B], bf16, kind="ExternalInput").ap()
    xl_d = nc.dram_tensor("xl", [I, B], f16, kind="ExternalInput").ap()
    wq_d = nc.dram_tensor("wq", [I, 2 * CN], f16, kind="ExternalInput").ap()
    corr_d = nc.dram_tensor("corr", [P, CN], f32, kind="ExternalInput").ap()
    rev_d = nc.dram_tensor("revio", [P, CN], f32, kind="ExternalInput").ap()
    oh_d = nc.dram_tensor("oh", [B, CN], f32, kind="ExternalOutput").ap()

    with tile.TileContext(nc) as tc:
        with ExitStack() as ctx:
            cpool = ctx.enter_context(tc.tile_pool(name="const", bufs=1))
            xhp = ctx.enter_context(tc.tile_pool(name="xhp", bufs=12))
            xlp = ctx.enter_context(tc.tile_pool(name="xlp", bufs=12))
            wp = ctx.enter_context(tc.tile_pool(name="wp", bufs=10))
            ppool = ctx.enter_context(tc.tile_pool(name="ps", bufs=1, space="PSUM"))
            dpool = ctx.enter_context(tc.tile_pool(name="dv", bufs=1))
            opool = ctx.enter_context(tc.tile_pool(name="ohp", bufs=2))

            rev_t = cpool.tile([P, CN], f32)
            nc.sync.dma_start(rev_t[:], rev_d[:])
            corr_t = cpool.tile([P, CN], f32)
            nc.sync.dma_start(corr_t[:], corr_d[:])

            # Per b-tile: 2 k-split accumulators [128, 512] for the xh
            # passes (cols 0:256 = xh*wh, 256:512 = xh*wl), and one
            # [128, 256] accumulator for the xl*wh pass.
            am = [[ppool.tile([P, 2 * CN], f32, tag=f"am{bt}{q}",
                              name=f"am{bt}{q}") for q in range(2)]
                  for bt in range(2)]
            al = [ppool.tile([P, CN], f32, tag=f"al{bt}", name=f"al{bt}")
                  for bt in range(2)]

            for it in range(NKC // G):
                xhg = xhp.tile([P, G, B], bf16)
                nc.gpsimd.dma_start(
                    xhg[:],
                    xh_d[it * G * KC:(it + 1) * G * KC, :]
                    .rearrange("(p g) j -> p g j", g=G))
                xlg = xlp.tile([P, G, B], f16)
                nc.gpsimd.dma_start(
                    xlg[:],
                    xl_d[it * G * KC:(it + 1) * G * KC, :]
                    .rearrange("(p g) j -> p g j", g=G))
                wg = wp.tile([P, G, 2 * CN], f16)
                nc.sync.dma_start(
                    wg[:],
                    wq_d[it * G * KC:(it + 1) * G * KC, :]
                    .rearrange("(p g) j -> p g j", g=G))
                for g in range(G):
                    kc = it * G + g
                    q, pos = divmod(kc, NKC // 2)
                    for bt in range(2):
                        bs = slice(bt * P, (bt + 1) * P)
                        nc.tensor.matmul(
                            am[bt][q][:],
                            lhsT=xhg[:, g, bs], rhs=wg[:, g, :],
                            start=(pos == 0), stop=(pos == NKC // 2 - 1))
                        nc.tensor.matmul(
                            al[bt][:],
                            lhsT=xlg[:, g, bs], rhs=wg[:, g, 0:CN],
                            start=(kc == 0), stop=(kc == NKC - 1))

            for bt in range(2):
                # Combine; never two PSUM operands in one op.
                c0 = dpool.tile([P, 2 * CN], f32, tag="c0")
                nc.vector.tensor_copy(c0[:], am[bt][0][:])
                a1 = dpool.tile([P, 2 * CN], f32, tag="a1")
                nc.vector.tensor_add(a1[:], c0[:], am[bt][1][:])
                # sw = xh*wh + 2^-10 * xh*wl
                sw = dpool.tile([P, CN], f32, tag="sw")
                nc.vector.scalar_tensor_tensor(
                    sw[:], a1[:, CN:2 * CN], 1.0 / WLS, a1[:, 0:CN],
                    op0=mybir.AluOpType.mult,
                    op1=mybir.AluOpType.add)
                sx = dpool.tile([P, CN], f32, tag="sx")
                nc.vector.scalar_tensor_tensor(
                    sx[:], al[bt][:], 1.0 / XLS, sw[:],
                    op0=mybir.AluOpType.mult,
                    op1=mybir.AluOpType.add)
                s_t = dpool.tile([P, CN], f32, tag="s")
                nc.vector.tensor_add(s_t[:], sx[:], corr_t[:])

                s3 = s_t[:].rearrange("p (s j) -> p s j", s=CPC)
                maxs = dpool.tile([P, CPC], f32, tag="maxs")
                nc.vector.tensor_reduce(maxs[:], s3, mybir.AxisListType.X,
                                        mybir.AluOpType.max)
                t_t = dpool.tile([P, CN], f32, tag="tt")
                for s in range(CPC):
                    seg = slice(s * N, (s + 1) * N)
                    nc.vector.scalar_tensor_tensor(
                        t_t[:, seg], s_t[:, seg], maxs[:, s:s + 1],
                        rev_t[:, seg],
                        op0=mybir.AluOpType.is_equal,
                        op1=mybir.AluOpType.mult)
                m2 = dpool.tile([P, CPC], f32, tag="m2")
                nc.vector.tensor_reduce(
                    m2[:], t_t[:].rearrange("p (s j) -> p s j", s=CPC),
                    mybir.AxisListType.X, mybir.AluOpType.max)
                oh_t = opool.tile([P, CN], f32)
                for s in range(CPC):
                    seg = slice(s * N, (s + 1) * N)
                    nc.vector.tensor_scalar(
                        oh_t[:, seg], rev_t[:, seg], m2[:, s:s + 1], None,
                        op0=mybir.AluOpType.is_equal)
                nc.sync.dma_start(oh_d[bt * P:(bt + 1) * P, :], oh_t[:])

    nc.compile()
    return nc


def kernel(x, weights):
    global _compiled, LAST_RESULTS
    x = np.asarray(x, dtype=np.float32)
    w = np.asarray(weights, dtype=np.float32)

    xt = np.ascontiguousarray(x.reshape(B, I).T).astype(np.float64) - 0.5
    xh = xt.astype(ml_dtypes.bfloat16)
    xl = ((xt - xh.astype(np.float64)) * XLS).astype(np.float16)
    xh = np.ascontiguousarray(xh)
    xl = np.ascontiguousarray(xl)
    j = np.arange(N, dtype=np.float32)
    revio = np.ascontiguousarray(
        np.tile(N - j, (P, CPC)).astype(np.float32))        # [128, 256]

    in_maps = []
    for c in range(N_CORES):
        wt = np.ascontiguousarray(
            w[c * CPC:(c + 1) * CPC].transpose(1, 0, 2).reshape(I, CN))
        wc = wt.astype(np.float64) - 0.5
        wh = wc.astype(np.float16)
        wl = ((wc - wh.astype(np.float64)) * WLS).astype(np.float16)
        wq = np.ascontiguousarray(
            np.concatenate([wh, wl], axis=1))               # [I, 512] fp16
        csum = 0.5 * wc.sum(axis=0)                         # [256] exact
        corr = np.ascontiguousarray(
            np.tile(csum.astype(np.float32), (P, 1)))       # [128, 256]
        in_maps.append({"xh": xh, "xl": xl, "wq": wq, "corr": corr,
                        "revio": revio})

    if _compiled is None:
        _compiled = _build()

    import os
    kwargs = {}
    if os.environ.get("KERNEL_TRACE"):
        kwargs = {"trace": True,
                  "tmpdir": os.environ.get("KERNEL_TRACE_DIR") or None}
    res = bass_utils.run_bass_kernel_spmd(
        _compiled, in_maps, core_ids=list(range(N_CORES)), **kwargs)
    LAST_RESULTS = res

    out = np.concatenate(
        [res.results[c]["oh"].reshape(B, CPC, N) for c in range(N_CORES)],
        axis=1)
    return np.ascontiguousarray(out.astype(np.float32))
